# revision 2
# baseline (speedup 1.0000x reference)
"""AngularLayer Trainium2 kernel: [500000, 63] -> [500000, 483].

Per row: 21 (x,y) landmarks -> 210 ordered-pair unit direction vectors
(clipped x/y components), appended to the input row.

Sharded batch-parallel over 8 NeuronCores (62500 rows/core), SPMD one graph.
Layout per core: tiles of [125 partitions x R rows], features on the free
axis.

v2 compute graph (planar): pair differences are written PLANAR
([x-plane | y-plane] in a bf16 tile) instead of interleaved.  This makes
the norm-sum add and the two normalize multiplies fully dense bf16 ops,
which the DVE runs in 2x packed mode (vs 1x strided before).  rsqrt is
computed once (dense) instead of twice (strided).  GPSIMD does the final
clip + planar->interleaved + bf16->f32 convert into the output tile, and
also takes the small-np_i tail of the pair-difference subtractions to
offload the DVE (the overall bottleneck).
"""

import os
from contextlib import ExitStack

import numpy as np

import concourse.bass as bass
import concourse.mybir as mybir
import concourse.tile as tile
from concourse import bacc
from concourse.bass_utils import run_bass_kernel_spmd

F32 = mybir.dt.float32
BF16 = mybir.dt.bfloat16
AF = mybir.ActivationFunctionType
ALU = mybir.AluOpType

N_CORES = 8
B_FULL = 500000
B_SHARD = B_FULL // N_CORES  # 62500
PARTS = 125
NLM = 21
NPAIR = 210
IN_C = 63
OUT_C = 483

ROWS_PER_PART = int(os.environ.get("ANGULAR_R", "10"))
GP_K = int(os.environ.get("ANGULAR_GPK", "14"))  # subs with i >= GP_K go to GPSIMD
O_BUFS = int(os.environ.get("ANGULAR_OBUFS", "5"))


def _build_nc(b_shard: int, rows_per_part: int, gp_k: int) -> bass.Bass:
    R = rows_per_part
    assert b_shard % (PARTS * R) == 0
    n_tiles = b_shard // (PARTS * R)
    NQ = R * NPAIR  # per-partition pair count (one plane)

    nc = bacc.Bacc("TRN2", target_bir_lowering=False, debug=False)
    inp = nc.dram_tensor("tensor", [b_shard, IN_C], F32, kind="ExternalInput")
    outp = nc.dram_tensor("out", [b_shard, OUT_C], F32, kind="ExternalOutput")

    with tile.TileContext(nc) as tc, ExitStack() as ctx:
        opool = ctx.enter_context(tc.tile_pool(name="o", bufs=O_BUFS))
        vpool = ctx.enter_context(tc.tile_pool(name="vxy", bufs=3))
        sqxp = ctx.enter_context(tc.tile_pool(name="sqx", bufs=2))
        npool = ctx.enter_context(tc.tile_pool(name="nsq", bufs=2))
        rrpool = ctx.enter_context(tc.tile_pool(name="rr", bufs=2))
        tpool = ctx.enter_context(tc.tile_pool(name="tt", bufs=2))

        # per-tile state carried across software-pipeline stages
        st: dict = {}

        def stage_a(t):
            # DMA in + pair differences (planar: [2, R, 210] = x plane | y plane)
            base = t * PARTS * R
            o = opool.tile([PARTS, R * OUT_C], F32, tag="o")
            o3 = o[:].rearrange("p (r c) -> p r c", c=OUT_C)

            # input loads into output tile's first 63 cols, both HWDGE
            # queues.  The runtime splits each DMA over E = largest
            # divisor(partition count) <= 16 SDMA engines -> use 60/64/1.
            src = inp[base:base + PARTS * R, :].rearrange(
                "(p r) c -> p r c", p=PARTS)
            nc.sync.dma_start(out=o3[0:60, :, 0:IN_C], in_=src[0:60].opt())
            nc.scalar.dma_start(out=o3[60:124, :, 0:IN_C], in_=src[60:124].opt())
            nc.sync.dma_start(out=o3[124:125, :, 0:IN_C], in_=src[124:125].opt())

            # pair differences, planar out: vxy[:, 0:NQ] = x diffs,
            # vxy[:, NQ:2NQ] = y diffs, each [r, q] row-major in q order.
            vxy = vpool.tile([PARTS, 2 * NQ], BF16, tag="vxy")
            vp = vxy[:].rearrange("p (two r q) -> p two r q", two=2, r=R)
            pb = 0
            for i in range(NLM - 1):
                np_i = NLM - 1 - i
                # minuend: landmarks i+1..20, component c -> [p, 2, r, np_i]
                minu = o3[:, :, 3 * (i + 1):IN_C].rearrange(
                    "p r (k three) -> p three r k", three=3)[:, 0:2, :, :]
                subt = o3[:, :, 3 * i:3 * i + 2].rearrange(
                    "p r two -> p two r").unsqueeze(3).broadcast_to(
                    (PARTS, 2, R, np_i))
                dst = vp[:, :, :, pb:pb + np_i]
                if i >= gp_k:
                    nc.gpsimd.tensor_sub(dst, minu, subt)
                else:
                    nc.vector.tensor_sub(dst, minu, subt)
                pb += np_i
            st[t] = {"o": o, "o3": o3, "vxy": vxy}

        def stage_b(t):
            # squares (dense) -> nsq (dense 2x) -> rsqrt (dense, once)
            # -> tilts = vxy * rr (two dense 2x plane mults)
            vxy = st[t]["vxy"]
            sq = sqxp.tile([PARTS, 2 * NQ], BF16, tag="sqx")
            nc.scalar.activation(sq[:], vxy[:], AF.Square)

            nsq = npool.tile([PARTS, NQ], BF16, tag="nsq")
            nc.vector.tensor_add(nsq[:], sq[:, 0:NQ], sq[:, NQ:2 * NQ])

            rr = rrpool.tile([PARTS, NQ], BF16, tag="rr")
            nc.scalar.activation(rr[:], nsq[:], AF.Abs_reciprocal_sqrt)

            tt = tpool.tile([PARTS, 2 * NQ], BF16, tag="tt")
            nc.vector.tensor_mul(tt[:, 0:NQ], vxy[:, 0:NQ], rr[:])
            nc.vector.tensor_mul(tt[:, NQ:2 * NQ], vxy[:, NQ:2 * NQ], rr[:])
            st[t]["tt"] = tt

        def stage_c(t):
            # clip + planar->interleaved + bf16->f32 [GPSIMD], DMA out
            base = t * PARTS * R
            o, o3, tt = (st[t][k] for k in ("o", "o3", "tt"))
            o_tilt = o3[:, :, IN_C:OUT_C].rearrange(
                "p r (q two) -> p two r q", two=2)
            tp = tt[:].rearrange("p (two r q) -> p two r q", two=2, r=R)
            nc.gpsimd.tensor_scalar(o_tilt, tp, 1.0, -1.0, ALU.min, ALU.max)

            dst = outp[base:base + PARTS * R, :].rearrange(
                "(p r) c -> p (r c)", p=PARTS)
            nc.sync.dma_start(out=dst[0:60], in_=o[0:60, :])
            nc.scalar.dma_start(out=dst[60:124], in_=o[60:124, :])
            nc.sync.dma_start(out=dst[124:125], in_=o[124:125, :])
            del st[t]

        # 3-stage software pipeline; issue oldest tile's drain first so the
        # GPSIMD clip + DMA-out aren't queued behind newer tiles' work.
        for s in range(n_tiles + 2):
            if s >= 2:
                stage_c(s - 2)
            if s < n_tiles:
                stage_a(s)
            if 1 <= s <= n_tiles:
                stage_b(s - 1)

    nc.compile()
    return nc


_NC_CACHE: dict = {}


def _get_nc():
    key = (B_SHARD, ROWS_PER_PART, GP_K)
    if key not in _NC_CACHE:
        _NC_CACHE[key] = _build_nc(B_SHARD, ROWS_PER_PART, GP_K)
    return _NC_CACHE[key]


def kernel(tensor: np.ndarray) -> np.ndarray:
    tensor = np.ascontiguousarray(np.asarray(tensor, dtype=np.float32))
    assert tensor.shape == (B_FULL, IN_C), tensor.shape

    nc = _get_nc()
    in_maps = [
        {"tensor": tensor[c * B_SHARD:(c + 1) * B_SHARD]} for c in range(N_CORES)
    ]
    trace = os.environ.get("ANGULAR_TRACE", "0") == "1"
    res = run_bass_kernel_spmd(
        nc, in_maps, core_ids=list(range(N_CORES)), trace=trace
    )
    if trace:
        kernel.last_exec_time_ns = res.exec_time_ns
        kernel.last_results = res
    out = np.concatenate([res.results[c]["out"] for c in range(N_CORES)], axis=0)
    return out
